# revision 13
# baseline (speedup 1.0000x reference)
"""Trainium2 Bass kernel for the NeuralMartingale stochastic-integral problem.

si[g] = sum_{j,m} (nn(t_m, x_m + s_j) - nn(t_m, x_m))[g] * R_delta[m,j] * valid[j,m]

Reformulated as a single weighted sum over a flat row list:
  - each valid displaced state (j,m) is a row with weight R_delta[m,j]
  - each base state m is a row with weight -sum_j R_delta[m,j]*valid[j,m]
  - rows are kept in m-groups (displaced rows of m, then base row of m) so the
    running fp32 accumulation stays O(|si|) instead of O(200x|si|)
  - the RNN encoding eY (constant across rows) is folded into the backbone
    bias, shrinking the big matmul contraction from 1025 to 514 features
  - rows with weight 0 (invalid displacements) are dropped entirely

m-groups are distributed round-robin over the 8 NeuronCores; each core runs an
identical program over [128, T*512] feature-major inputs and returns a partial
[64] sum; the host adds the 8 partials.
"""

import sys

for _p in ("/opt/trn_rl_repo", "/root/.axon_site/_ro/trn_rl_repo"):
    if _p not in sys.path:
        sys.path.append(_p)

import numpy as np

import concourse.bass as bass
import concourse.tile as tile
import concourse.mybir as mybir
from concourse import bacc
from concourse.bass_utils import run_bass_kernel_spmd

F32 = mybir.dt.float32
F32R = mybir.dt.float32r
N_CORES = 8
P = 128
NTILE = 512  # moving-dim per row tile (fp32 max)

# architecture constants of this nn.Module instance
DX, MR, GOUT = 16, 64, 64
HX, EX, HRNN, HB = 1024, 512, 512, 2048

LAST_RESULTS = None  # BassKernelResults of the most recent run (for test.py)

_PROGRAM_CACHE: dict = {}


def r32r(a):
    """fp32 -> fp32r value rounding (11-bit mantissa, round-to-nearest-even).

    fp32r storage is fp32 bits with the low 12 mantissa bits zero; matmul
    operands must carry this dtype, so fp32r DRAM tensors are pre-rounded here.
    """
    u = np.ascontiguousarray(np.asarray(a, np.float32)).view(np.uint32)
    lsb = (u >> 12) & 1
    u = (u + np.uint32(0x7FF) + lsb) & np.uint32(0xFFFFF000)
    return u.view(np.float32)


def _build_program(T: int):
    """Bass program for one core: 4-layer MLP over T row-tiles of 512 rows,
    feature-major activations, fp32r matmuls, weighted accumulation to [64,1].
    """
    NR = T * NTILE
    nc = bacc.Bacc("TRN2", target_bir_lowering=False, debug=False)

    xin_d = nc.declare_dram_parameter("xin", [P, NR], F32R, isOutput=False)
    extra_d = nc.declare_dram_parameter("extra", [P, NR], F32R, isOutput=False)
    wb_d = nc.declare_dram_parameter("wb", [GOUT, NR], F32, isOutput=False)
    wx1_d = nc.declare_dram_parameter("wx1", [P, HX], F32R, isOutput=False)
    bx1_d = nc.declare_dram_parameter("bx1", [P, HX // P], F32, isOutput=False)
    wx2_d = nc.declare_dram_parameter("wx2", [P, (HX // P) * EX], F32R, isOutput=False)
    bx2_d = nc.declare_dram_parameter("bx2", [P, EX // P], F32, isOutput=False)
    wb1_d = nc.declare_dram_parameter("wb1", [P, 5 * HB], F32R, isOutput=False)
    wb2_d = nc.declare_dram_parameter("wb2", [P, (HB // P) * GOUT], F32R, isOutput=False)
    out_d = nc.declare_dram_parameter("out", [GOUT, 1], F32, isOutput=True)

    KX1 = HX // P      # 8  M-tiles of layer 1
    KX2 = HX // P      # 8  K-tiles of layer 2
    MX2 = EX // P      # 4  M-tiles of layer 2
    KB1 = 5            # K-tiles of backbone 1 (4 x eX + 1 extra[t, 1, 0...])
    MB1 = HB // P      # 16 M-tiles of backbone 1
    KB2 = HB // P      # 16 K-tiles of backbone 2

    Tanh = mybir.ActivationFunctionType.Tanh
    Ident = mybir.ActivationFunctionType.Identity

    from contextlib import ExitStack

    with tile.TileContext(nc) as tc, ExitStack() as ctx:
        wpool = ctx.enter_context(tc.tile_pool(name="weights", bufs=1))
        xpool = ctx.enter_context(tc.tile_pool(name="acts_in", bufs=3))
        h1pool = ctx.enter_context(tc.tile_pool(name="h1", bufs=2))
        expool = ctx.enter_context(tc.tile_pool(name="ex", bufs=2))
        h2pool = ctx.enter_context(tc.tile_pool(name="h2", bufs=2))
        pspool = ctx.enter_context(tc.tile_pool(name="ps", bufs=8, space="PSUM"))
        accpool = ctx.enter_context(tc.tile_pool(name="acc", bufs=T + 2))
        scpool = ctx.enter_context(tc.tile_pool(name="scratch", bufs=2))

        # weight loads (emitted in order of first use)
        wx1_t = wpool.tile([P, HX], F32R, tag="wx1")
        nc.sync.dma_start(wx1_t[:], wx1_d[:])
        bx1_t = wpool.tile([P, KX1], F32, tag="bx1")
        nc.sync.dma_start(bx1_t[:], bx1_d[:])
        wx2_t = wpool.tile([P, KX2 * EX], F32R, tag="wx2")
        nc.sync.dma_start(wx2_t[:], wx2_d[:])
        bx2_t = wpool.tile([P, MX2], F32, tag="bx2")
        nc.sync.dma_start(bx2_t[:], bx2_d[:])
        wb1_t = wpool.tile([P, 5 * HB], F32R, tag="wb1")
        nc.sync.dma_start(wb1_t[:], wb1_d[:])
        wb2_t = wpool.tile([P, KB2 * GOUT], F32R, tag="wb2")
        nc.sync.dma_start(wb2_t[:], wb2_d[:])

        acc = accpool.tile([GOUT, 1], F32, tag="acc")
        nc.vector.memset(acc[:], 0.0)

        for rt in range(T):
            sl = bass.ts(rt, NTILE)
            xin_t = xpool.tile([P, NTILE], F32R, tag="xin")
            nc.scalar.dma_start(xin_t[:], xin_d[:, sl])
            extra_t = xpool.tile([P, NTILE], F32R, tag="extra")
            nc.scalar.dma_start(extra_t[:], extra_d[:, sl])
            wb_t = xpool.tile([GOUT, NTILE], F32, tag="wb")
            nc.scalar.dma_start(wb_t[:], wb_d[:, sl])

            # layer 1: h1 = tanh(Wx1^T x + bx1), K=128 zero-padded, 8 M-tiles.
            # partitions 0:16 carry x against r32r(Wx1); partitions 16:32 carry
            # x again against the rounding residual -> exact fp32 layer 1.
            h1 = []
            for mi in range(KX1):
                ps = pspool.tile([P, NTILE], F32, tag="ps")
                nc.tensor.matmul(
                    ps[:],
                    wx1_t[:, bass.ts(mi, P)],
                    xin_t[:],
                    start=True,
                    stop=True,
                )
                ht = h1pool.tile([P, NTILE], F32R, tag=f"h1_{mi}")
                nc.scalar.activation(ht[:], ps[:], Tanh, bias=bx1_t[:, mi : mi + 1])
                h1.append(ht)

            # layer 2: eX = Wx2^T h1 + bx2, K=1024 (8 tiles), 4 M-tiles
            ex = []
            for mi in range(MX2):
                ps = pspool.tile([P, NTILE], F32, tag="ps")
                for ki in range(KX2):
                    nc.tensor.matmul(
                        ps[:],
                        wx2_t[:, EX * ki + P * mi : EX * ki + P * mi + P],
                        h1[ki][:],
                        start=(ki == 0),
                        stop=(ki == KX2 - 1),
                    )
                ext = expool.tile([P, NTILE], F32R, tag=f"ex_{mi}")
                nc.scalar.activation(ext[:], ps[:], Ident, bias=bx2_t[:, mi : mi + 1])
                ex.append(ext)
            ex.append(extra_t)  # K-tile 4: [t; 1; zeros] rows (bias+time features)

            # backbone 1: h2 = tanh(Wb1_ext^T [eX; t; 1]), K=640 (5 tiles), 16 M-tiles
            h2 = []
            for mo in range(MB1):
                ps = pspool.tile([P, NTILE], F32, tag="ps")
                for ki in range(KB1):
                    nc.tensor.matmul(
                        ps[:],
                        wb1_t[:, HB * ki + P * mo : HB * ki + P * mo + P],
                        ex[ki][:],
                        start=(ki == 0),
                        stop=(ki == KB1 - 1),
                    )
                h2t = h2pool.tile([P, NTILE], F32R, tag=f"h2_{mo}")
                nc.scalar.activation(h2t[:], ps[:], Tanh)
                h2.append(h2t)

            # backbone 2: o = Wb2^T h2 (no bias; folded to host), K=2048 (16 tiles)
            ps2 = pspool.tile([GOUT, NTILE], F32, tag="ps")
            for ki in range(KB2):
                nc.tensor.matmul(
                    ps2[:],
                    wb2_t[:, GOUT * ki : GOUT * ki + GOUT],
                    h2[ki][:],
                    start=(ki == 0),
                    stop=(ki == KB2 - 1),
                )

            # weighted accumulation: acc += sum_r wb[:, r] * o[:, r]
            # (fused tensor_tensor_reduce with PSUM in0 crashes TRN2; decompose)
            prod = scpool.tile([GOUT, NTILE], F32, tag="prod")
            nc.vector.tensor_mul(prod[:], ps2[:], wb_t[:])
            red = scpool.tile([GOUT, 1], F32, tag="red")
            nc.vector.reduce_sum(red[:], prod[:], axis=mybir.AxisListType.X)
            acc_new = accpool.tile([GOUT, 1], F32, tag="acc")
            nc.vector.tensor_add(acc_new[:], acc[:], red[:])
            acc = acc_new

        nc.sync.dma_start(out_d[:], acc[:])

    nc.compile()
    return nc


def prepare(X, Y, R, stoich, times_t, times_tau,
            Wx1, bx1, Wx2, bx2, Wih, Whh, bh, Wb1, bb1, Wb2, bb2,
            k, k_prime, q, q_prime):
    """Host-side prep: returns (nc, in_maps, tot_w, bb2) ready for SPMD launch."""
    X = np.asarray(X, np.float32)
    Y = np.asarray(Y, np.float32)
    R = np.asarray(R, np.float32)
    stoich = np.asarray(stoich, np.float32)
    times_t = np.asarray(times_t, np.float32)
    times_tau = np.asarray(times_tau, np.float32)
    Wx1 = np.asarray(Wx1, np.float32); bx1 = np.asarray(bx1, np.float32)
    Wx2 = np.asarray(Wx2, np.float32); bx2 = np.asarray(bx2, np.float32)
    Wih = np.asarray(Wih, np.float32); Whh = np.asarray(Whh, np.float32)
    bh = np.asarray(bh, np.float32)
    Wb1 = np.asarray(Wb1, np.float32); bb1 = np.asarray(bb1, np.float32)
    Wb2 = np.asarray(Wb2, np.float32); bb2 = np.asarray(bb2, np.float32)
    k = int(k); k_prime = int(k_prime); q = int(q); q_prime = int(q_prime)

    m_bar = times_tau.shape[0]
    base = k_prime * m_bar

    # ---- host: Elman RNN encoder over Y slice (tiny; constant across rows) ----
    h = np.zeros(HRNN, np.float32)
    for yrow in Y[q, k + 1 :, :]:
        h = np.tanh(yrow @ Wih + h @ Whh + bh).astype(np.float32)
    eY = h
    bb1_eff = (bb1 + eY @ Wb1[1 + EX :, :]).astype(np.float32)

    # ---- host: flat row list in m-groups ----
    Xbase = X[q_prime, base : base + m_bar]                      # [m_bar, DX]
    t = (times_t[k] + times_tau).astype(np.float32)              # [m_bar]
    X_disp = Xbase[None, :, :] + stoich.T[:, None, :]            # [MR, m_bar, DX]
    valid = (X_disp >= 0).all(-1)                                # [MR, m_bar]
    R_delta = R[q_prime, base + 1 : base + m_bar + 1, :] - R[q_prime, base : base + m_bar, :]
    w_disp = (R_delta.T * valid).astype(np.float32)              # [MR, m_bar]
    c_m = w_disp.sum(axis=0)                                     # [m_bar]

    core_rows_x = [[] for _ in range(N_CORES)]
    core_rows_t = [[] for _ in range(N_CORES)]
    core_rows_w = [[] for _ in range(N_CORES)]
    for m in range(m_bar):
        c = m % N_CORES
        js = np.nonzero(w_disp[:, m])[0]
        if len(js):
            core_rows_x[c].append(X_disp[js, m])
            core_rows_t[c].append(np.full(len(js), t[m], np.float32))
            core_rows_w[c].append(w_disp[js, m])
        core_rows_x[c].append(Xbase[m : m + 1])
        core_rows_t[c].append(np.full(1, t[m], np.float32))
        core_rows_w[c].append(np.array([-c_m[m]], np.float32))

    per_core = []
    max_rows = 0
    for c in range(N_CORES):
        rx = np.concatenate(core_rows_x[c]).astype(np.float32)
        rt_ = np.concatenate(core_rows_t[c]).astype(np.float32)
        rw = np.concatenate(core_rows_w[c]).astype(np.float32)
        per_core.append((rx, rt_, rw))
        max_rows = max(max_rows, len(rw))

    T = max(1, -(-max_rows // NTILE))
    NR = T * NTILE

    # ---- host: weight repacking (feature-major, K-tiled, fp32r pre-rounded) ----
    # layer-1 split: rows 0:16 hold r32r(Wx1), rows 16:32 hold the rounding
    # residual; xin duplicates x in partitions 16:32 -> layer 1 is exact fp32.
    wx1_p = np.zeros((P, HX), np.float32)
    wx1_hi = r32r(Wx1)
    wx1_p[:DX] = wx1_hi
    wx1_p[DX : 2 * DX] = r32r(Wx1 - wx1_hi)
    bx1_c = np.ascontiguousarray(bx1.reshape(HX // P, P).T)
    wx2_c = r32r(np.ascontiguousarray(
        Wx2.reshape(HX // P, P, EX).transpose(1, 0, 2).reshape(P, -1)))
    bx2_c = np.ascontiguousarray(bx2.reshape(EX // P, P).T)
    wb1_ext = np.zeros((5 * P, HB), np.float32)
    wb1_ext[:EX] = Wb1[1 : 1 + EX]
    wb1_ext[EX] = Wb1[0]
    wb1_ext[EX + 1] = bb1_eff
    wb1_c = r32r(np.ascontiguousarray(
        wb1_ext.reshape(5, P, HB).transpose(1, 0, 2).reshape(P, -1)))
    wb2_c = r32r(np.ascontiguousarray(
        Wb2.reshape(HB // P, P, GOUT).transpose(1, 0, 2).reshape(P, -1)))

    shared = dict(wx1=wx1_p, bx1=bx1_c, wx2=wx2_c, bx2=bx2_c, wb1=wb1_c, wb2=wb2_c)

    in_maps = []
    for c in range(N_CORES):
        rx, rt_, rw = per_core[c]
        n = len(rw)
        xin = np.zeros((P, NR), np.float32)
        xin[:DX, :n] = rx.T            # integer-valued: exact in fp32r
        xin[DX : 2 * DX, :n] = rx.T    # duplicate for the Wx1-residual rows
        extra = np.zeros((P, NR), np.float32)
        extra[0, :n] = r32r(rt_)
        extra[1, :] = 1.0
        wb = np.zeros((GOUT, NR), np.float32)
        wb[:, :n] = np.broadcast_to(rw, (GOUT, n))
        in_maps.append(dict(xin=xin, extra=extra, wb=wb, **shared))

    if T not in _PROGRAM_CACHE:
        _PROGRAM_CACHE[T] = _build_program(T)
    nc = _PROGRAM_CACHE[T]

    # exact-math sum of all weights is 0; the fp32 residue times bb2 is added back
    tot_w = np.float32(sum(np.sum(rw, dtype=np.float64) for _, _, rw in per_core))
    return nc, in_maps, tot_w, bb2


def kernel(**inputs):
    global LAST_RESULTS
    nc, in_maps, tot_w, bb2 = prepare(**inputs)
    res = run_bass_kernel_spmd(nc, in_maps, list(range(N_CORES)))
    LAST_RESULTS = res

    si = np.zeros(GOUT, np.float32)
    for c in range(N_CORES):
        si = si + res.results[c]["out"].reshape(GOUT)
    si = (si + tot_w * bb2).astype(np.float32)
    return si
